# revision 6
# baseline (speedup 1.0000x reference)
"""ALiBi multi-head attention on 8 TRN2 NeuronCores.

Sharding: data-parallel over batch (B=2 -> 2 groups of 4 cores), tensor-parallel
over the 16 heads (4 heads per core, Megatron-style column shards of Wq/Wk/Wv).
The output projection uses an AllGather of the per-head attention outputs inside
each 4-core group followed by a column shard of Wo on every core (cheaper than
the row-shard + AllReduce formulation: 1MB gather vs 8MB reduce per core).

Head assignment is slot-ranked: core group-position j gets heads
{15-j, 11-j, 7-j, 3-j}. All cores run one SPMD instruction stream, so the
ALiBi band schedule of slot s is sized for the widest head in that slot; the
per-core ALiBi factor tiles F = exp(-slope*|k-q|) (fp16, Toeplitz: one tile per
tile-diagonal offset) carry each core's actual slopes and double as the band
mask (F underflows to exactly 0 outside the head's own band).

On-chip layout: activations are passed transposed ([D, S]) so the Q/K
projections directly produce Q^T/K^T ([head_dim, S]) — the layout needed by
scoresT = (K^T).T @ Q^T — while V is produced in natural [S, head_dim] layout
with a fused ones-column so the attention-value matmul also emits the softmax
denominators (row 64) for free. Softmax skips the max-subtraction: for this
problem's scale (scores ~ N(0, 0.45^2), max < 3) fp32 exp cannot overflow.
"""

import math
import sys

import numpy as np

if "/opt/trn_rl_repo" not in sys.path:
    sys.path.insert(0, "/opt/trn_rl_repo")

import concourse.bass as bass  # noqa: E402
import concourse.mybir as mybir  # noqa: E402
import concourse.tile as tile  # noqa: E402
from concourse import bacc  # noqa: E402
from concourse.bass_utils import run_bass_kernel_spmd  # noqa: E402

B, S, D, H, HD = 2, 2048, 1024, 16, 64
NCORES = 8
GROUP = 4          # cores per batch group
NSLOT = 4          # head slots per core
KT = 128           # k (key position) tile size
NKT = S // KT      # 16
NQT = S // KT      # 16 q tiles of 128
NDC = D // 128     # 8 contraction chunks
T_FACTOR = 30.0    # band radius = T_FACTOR / slope  (exp(-30) ~ 1e-13)

F16 = mybir.dt.float16
F32 = mybir.dt.float32

SLOPES = [2.0 ** (-0.5 * (h + 1)) for h in range(H)]
# BINS[j][s] = head of slot s on cores j and j+4
BINS = [[15 - j, 11 - j, 7 - j, 3 - j] for j in range(GROUP)]


def _slot_nd():
    """Max |k_tile - q_tile| included per slot (widest head in the slot)."""
    nds = []
    for s in range(NSLOT):
        t = max(
            min(S - 1, int(math.ceil(T_FACTOR / SLOPES[BINS[j][s]])))
            for j in range(GROUP)
        )
        nds.append(min(NKT - 1, (t + KT - 1) // KT))
    return nds


SLOT_ND = _slot_nd()
F_BASE = []
_acc = 0
for _s in range(NSLOT):
    F_BASE.append(_acc)
    _acc += 2 * SLOT_ND[_s] + 1
NF = _acc  # total ALiBi factor tiles per core


def _f_idx(s, dd):
    """Index of the F tile for slot s, diagonal offset dd = k_tile - q_tile.

    Stored so that for fixed k_tile, consecutive q_tiles read consecutive
    F tiles (lets one tensor_mul cover a whole q-chunk).
    """
    return F_BASE[s] + SLOT_ND[s] - dd


def _chunks(qlo, qhi):
    """Split q-tile range [qlo, qhi] into chunks of <= 8 tiles."""
    w = qhi - qlo + 1
    out = []
    q = qlo
    while w > 0:
        c = min(8, w)
        out.append((q, c))
        q += c
        w -= c
    return out


def build_graph():
    nc = bacc.Bacc("TRN2", target_bir_lowering=False, debug=False,
                   num_devices=NCORES)
    xq = nc.dram_tensor("xq", [128, NDC * S], F16, kind="ExternalInput")
    xk = nc.dram_tensor("xk", [128, NDC * S], F16, kind="ExternalInput")
    xv = nc.dram_tensor("xv", [128, NDC * S], F16, kind="ExternalInput")
    wq = nc.dram_tensor("wq", [128, NDC * 256], F16, kind="ExternalInput")
    wk = nc.dram_tensor("wk", [128, NDC * 256], F16, kind="ExternalInput")
    wv = nc.dram_tensor("wv", [128, NDC * 256], F16, kind="ExternalInput")
    wo = nc.dram_tensor("wo", [128, NDC * 256], F16, kind="ExternalInput")
    fm = nc.dram_tensor("fm", [128, NF * KT], F16, kind="ExternalInput")
    out = nc.dram_tensor("out", [S, 256], F32, kind="ExternalOutput")

    with tile.TileContext(nc) as tc:
        with (
            tc.tile_pool(name="wpool", bufs=1) as wpool,
            tc.tile_pool(name="persist", bufs=1) as persist,
            tc.tile_pool(name="dram", bufs=1, space="DRAM") as dram,
        ):
            wq_sb = wpool.tile([128, NDC * 256], F16)
            wk_sb = wpool.tile([128, NDC * 256], F16)
            wv_sb = wpool.tile([128, NDC * 256], F16)
            wo_sb = wpool.tile([128, NDC * 256], F16)
            f_sb = wpool.tile([128, NF * KT], F16)
            nc.sync.dma_start(wq_sb[:], wq[:])
            nc.sync.dma_start(wk_sb[:], wk[:])
            nc.sync.dma_start(wv_sb[:], wv[:])
            nc.sync.dma_start(wo_sb[:], wo[:])
            nc.sync.dma_start(f_sb[:], fm[:])

            # Q^T/K^T: two slots per 128-partition tile (slot s%2 at
            # partition 64*(s%2)); V natural layout with ones column per
            # (k_tile, slot) at vx[:, kt*260 + s*65 + 64].
            qt_sb = [persist.tile([128, S], F16, name=f"qt{m}") for m in range(2)]
            kt_sb = [persist.tile([128, S], F16, name=f"kt{m}") for m in range(2)]
            vx_sb = persist.tile([128, NKT * NSLOT * 65], F16)
            ones_sb = persist.tile([1, 64], F16)
            nc.vector.memset(ones_sb[:], 1.0)
            nc.vector.memset(
                vx_sb[:].rearrange("p (k s e) -> p k s e", k=NKT, s=NSLOT)[:, :, :, 64:65],
                1.0,
            )
            normt_sb = [persist.tile([64, S], F16, name=f"nt{s}") for s in range(NSLOT)]

            # ---- projections -------------------------------------------------
            with (
                tc.tile_pool(name="xpool", bufs=1) as xpool,
                tc.tile_pool(name="prps", bufs=4, space="PSUM") as prps,
                tc.tile_pool(name="prsb", bufs=4) as prsb,
            ):
                xq_sb = xpool.tile([128, NDC * S], F16)
                xk_sb = xpool.tile([128, NDC * S], F16)
                xv_sb = xpool.tile([128, NDC * S], F16)
                nc.sync.dma_start(xq_sb[:], xq[:])
                nc.sync.dma_start(xk_sb[:], xk[:])
                nc.sync.dma_start(xv_sb[:], xv[:])

                for proj, (wsb, xsb, dsts) in enumerate(
                    (
                        (wq_sb, xq_sb, qt_sb),
                        (wk_sb, xk_sb, kt_sb),
                    )
                ):
                    for mb in range(2):  # slot pair
                        for nb in range(4):  # 512 wide seq block
                            ps = prps.tile([128, 512], F32, tag="pr")
                            for c in range(NDC):
                                nc.tensor.matmul(
                                    ps[:],
                                    lhsT=wsb[:, c * 256 + mb * 128:c * 256 + mb * 128 + 128],
                                    rhs=xsb[:, c * S + nb * 512:c * S + nb * 512 + 512],
                                    start=(c == 0),
                                    stop=(c == NDC - 1),
                                )
                            nc.vector.tensor_copy(
                                dsts[mb][:, nb * 512:(nb + 1) * 512], ps[:]
                            )

                for kt in range(NKT):
                    ps = prps.tile([128, 256], F32, tag="prv")
                    for c in range(NDC):
                        nc.tensor.matmul(
                            ps[:],
                            lhsT=xv_sb[:, c * S + kt * 128:c * S + kt * 128 + 128],
                            rhs=wv_sb[:, c * 256:(c + 1) * 256],
                            start=(c == 0),
                            stop=(c == NDC - 1),
                        )
                    for s in range(NSLOT):
                        nc.vector.tensor_copy(
                            vx_sb[:, kt * 260 + s * 65:kt * 260 + s * 65 + 64],
                            ps[:, s * 64:(s + 1) * 64],
                        )

            # ---- attention ---------------------------------------------------
            with (
                tc.tile_pool(name="scps", bufs=2, space="PSUM") as scps,
                tc.tile_pool(name="avps", bufs=4, space="PSUM") as avps,
                tc.tile_pool(name="psb", bufs=3) as psb,
                tc.tile_pool(name="nsb", bufs=4) as nsb,
            ):
                for s in range(NSLOT):
                    nd = SLOT_ND[s]
                    po = 64 * (s % 2)
                    qt_t = qt_sb[s // 2]
                    kt_t = kt_sb[s // 2]
                    av = [
                        avps.tile([65, 512], F32, tag="av", name=f"av{s}_{g}")
                        for g in range(4)
                    ]
                    # start/stop must be unique per av tile (2KB PSUM zero
                    # region): emission-ordered (kt, qb) pairs per group
                    sched = {g: [] for g in range(4)}
                    for kt in range(NKT):
                        for qb in range(max(0, kt - nd), min(NQT - 1, kt + nd) + 1):
                            sched[qb // 4].append((kt, qb))
                    first = {g: sched[g][0] for g in range(4)}
                    last = {g: sched[g][-1] for g in range(4)}
                    for kt in range(NKT):
                        qlo, qhi = max(0, kt - nd), min(NQT - 1, kt + nd)
                        for (q0, w) in _chunks(qlo, qhi):
                            sc = scps.tile([128, w * 128], F32, tag="sc")
                            for m0 in range(0, w, 4):
                                mw = min(4, w - m0)
                                nc.tensor.matmul(
                                    sc[:, m0 * 128:(m0 + mw) * 128],
                                    lhsT=kt_t[po:po + 64, kt * 128:kt * 128 + 128],
                                    rhs=qt_t[po:po + 64,
                                             (q0 + m0) * 128:(q0 + m0 + mw) * 128],
                                )
                            pt = psb.tile([128, w * 128], F16, tag="pt")
                            nc.scalar.activation(
                                pt[:], sc[:], mybir.ActivationFunctionType.Exp
                            )
                            fi = _f_idx(s, kt - q0)
                            nc.vector.tensor_mul(
                                pt[:], pt[:], f_sb[:, fi * 128:(fi + w) * 128]
                            )
                            for i in range(w):
                                qb = q0 + i
                                g = qb // 4
                                nc.tensor.matmul(
                                    av[g][:, (qb % 4) * 128:(qb % 4 + 1) * 128],
                                    lhsT=vx_sb[:, kt * 260 + s * 65:kt * 260 + s * 65 + 65],
                                    rhs=pt[:, i * 128:(i + 1) * 128],
                                    start=((kt, qb) == first[g]),
                                    stop=((kt, qb) == last[g]),
                                )
                    # normalize: rows/denominator live in av[g]
                    for g in range(4):
                        rec = nsb.tile([1, 512], F32, tag="rec")
                        nc.vector.reciprocal(rec[:], av[g][64:65, :])
                        rec16 = nsb.tile([1, 512], F16, tag="rec16")
                        nc.vector.tensor_copy(rec16[:], rec[:])
                        bc = scps.tile([64, 512], F32, tag="sc", name=f"bc{s}_{g}")
                        nc.tensor.matmul(bc[:], lhsT=ones_sb[:], rhs=rec16[:])
                        bc_sb = nsb.tile([64, 512], F16, tag="bcs")
                        nc.vector.tensor_copy(bc_sb[:], bc[:])
                        nc.vector.tensor_mul(
                            normt_sb[s][:, g * 512:(g + 1) * 512],
                            av[g][0:64, :],
                            bc_sb[:],
                        )

            # ---- all-gather + output projection ------------------------------
            agin = dram.tile([GROUP * 64, S], F16)
            agout = dram.tile([GROUP * 256, S], F16)
            for s in range(NSLOT):
                nc.sync.dma_start(agin[64 * s:64 * (s + 1), :], normt_sb[s][:])
            nc.gpsimd.collective_compute(
                "AllGather",
                mybir.AluOpType.bypass,
                ins=[agin.opt()],
                outs=[agout.opt()],
                replica_groups=[[0, 1, 2, 3], [4, 5, 6, 7]],
            )

            with (
                tc.tile_pool(name="gpool", bufs=1) as gpool,
                tc.tile_pool(name="ops", bufs=4, space="PSUM") as ops,
                tc.tile_pool(name="osb", bufs=4) as osb,
            ):
                gath = gpool.tile([128, NDC * S], F16)
                nc.sync.dma_start(
                    gath[:].rearrange("p (c t) -> p c t", c=NDC),
                    agout[:].rearrange("(c p) t -> p c t", p=128),
                )
                for m in range(16):
                    ps = ops.tile([128, 256], F32, tag="o")
                    for c in range(NDC):
                        nc.tensor.matmul(
                            ps[:],
                            lhsT=gath[:, c * S + m * 128:c * S + m * 128 + 128],
                            rhs=wo_sb[:, c * 256:(c + 1) * 256],
                            start=(c == 0),
                            stop=(c == NDC - 1),
                        )
                    ot = osb.tile([128, 256], F32, tag="ot")
                    nc.vector.tensor_copy(ot[:], ps[:])
                    nc.sync.dma_start(out[m * 128:(m + 1) * 128, :], ot[:])

    nc.compile()
    return nc


_NC_CACHE = None


def _get_graph():
    global _NC_CACHE
    if _NC_CACHE is None:
        _NC_CACHE = build_graph()
    return _NC_CACHE


def _swizzle_cd(a):
    """[C*128, X] -> [128, C*X] with row p holding chunks c at [c*X:(c+1)*X]."""
    c = a.shape[0] // 128
    return np.ascontiguousarray(
        a.reshape(c, 128, a.shape[1]).transpose(1, 0, 2).reshape(128, -1)
    )


def _host_inputs(query, key, value, Wq, Wk, Wv, Wo):
    xqs, xks, xvs = [], [], []
    for b in range(B):
        xqs.append(_swizzle_cd(query[b].T.astype(np.float32)).astype(np.float16))
        xks.append(_swizzle_cd(key[b].T.astype(np.float32)).astype(np.float16))
        xvs.append(_swizzle_cd(value[b].T.astype(np.float32)).astype(np.float16))

    scale = 1.0 / math.sqrt(HD)
    wqs, wks, wvs, fms = [], [], [], []
    for j in range(GROUP):
        cols = np.concatenate(
            [np.arange(64 * h, 64 * h + 64) for h in BINS[j]]
        )
        wqs.append(_swizzle_cd((Wq[:, cols] * scale).astype(np.float32)).astype(np.float16))
        wks.append(_swizzle_cd(Wk[:, cols].astype(np.float32)).astype(np.float16))
        wvs.append(_swizzle_cd(Wv[:, cols].astype(np.float32)).astype(np.float16))

        f = np.zeros((128, NF * KT), np.float32)
        p = np.arange(128)[:, None]
        q = np.arange(128)[None, :]
        for s in range(NSLOT):
            sl = SLOPES[BINS[j][s]]
            for dd in range(-SLOT_ND[s], SLOT_ND[s] + 1):
                fi = _f_idx(s, dd)
                f[:, fi * 128:(fi + 1) * 128] = np.exp(
                    -sl * np.abs(dd * 128 + p - q)
                )
        fms.append(f.astype(np.float16))

    # Wo rows permuted to gathered order: row 256*j + 64*s + d <-> 64*BINS[j][s]+d
    perm = np.concatenate(
        [np.arange(64 * BINS[j][s], 64 * BINS[j][s] + 64)
         for j in range(GROUP) for s in range(NSLOT)]
    )
    wos = []
    for j in range(GROUP):
        wos.append(
            _swizzle_cd(
                Wo[perm][:, 256 * j:256 * (j + 1)].astype(np.float32)
            ).astype(np.float16)
        )

    in_maps = []
    for i in range(NCORES):
        b, j = i // GROUP, i % GROUP
        in_maps.append({
            "xq": xqs[b], "xk": xks[b], "xv": xvs[b],
            "wq": wqs[j], "wk": wks[j], "wv": wvs[j], "wo": wos[j],
            "fm": fms[j],
        })
    return in_maps


def kernel(**inputs):
    query = np.asarray(inputs["query"], np.float32)
    key = np.asarray(inputs["key"], np.float32)
    value = np.asarray(inputs["value"], np.float32)
    Wq = np.asarray(inputs["Wq"], np.float32)
    Wk = np.asarray(inputs["Wk"], np.float32)
    Wv = np.asarray(inputs["Wv"], np.float32)
    Wo = np.asarray(inputs["Wo"], np.float32)

    nc = _get_graph()
    in_maps = _host_inputs(query, key, value, Wq, Wk, Wv, Wo)
    res = run_bass_kernel_spmd(nc, in_maps, list(range(NCORES)))

    full = np.empty((B, S, D), np.float32)
    for b in range(B):
        for j in range(GROUP):
            full[b][:, 256 * j:256 * (j + 1)] = res.results[GROUP * b + j]["out"]
    return full
